# revision 7
# baseline (speedup 1.0000x reference)
"""Trainium2 Bass kernel for a 3-layer LCN/GNN message-passing network.

Per layer: out[b, d] = relu(sum_k x[b, knn[d, k]] * w[d, k] + bias[d]),
then a small FC head: y = x3 @ fc_w.T + fc_b.

Strategy (data-parallel over batch across 8 cores, per sharding hint):
  - Each core owns 32 of the 256 samples. All index/weight tables are
    replicated; there is no cross-device traffic.
  - x is staged transposed (feature-major) so that one feature's 32-sample
    column is a contiguous 128B row in DRAM. The per-(node, k) gather is an
    indirect DMA: a (128, G*16) int32 index tile gathers G node-tiles'
    worth of rows into a (128, G*16, 32) SBUF tile in one instruction.
  - DVE does the in-place multiply by w and the k-reduction (AP-permuted so
    k is the innermost reduce axis); ACT applies bias+ReLU per node tile.
  - Layer outputs are written back to an internal DRAM table (the next
    layer's gather source). The final layer stays in SBUF and feeds PE
    matmuls contracting over nodes for the FC head.
"""

import os
import sys
import types

import numpy as np

# This container's axon client lacks the NTFF profile hook module; stub it
# so run_bass_kernel_spmd's trace path degrades to an untraced run instead
# of raising ImportError when BASS_TRACE is set in the environment.
try:  # pragma: no cover
    import antenv.axon_hooks  # noqa: F401
except Exception:
    _m = types.ModuleType("antenv.axon_hooks")
    _m.get_axon_ntff_profile_hook = lambda: None
    sys.modules["antenv.axon_hooks"] = _m

B, IN_DIM, K = 256, 16384, 16
DIMS = [8192, 4096, 2048]
OUT_DIM = 3
N_CORES = 8
BPC = B // N_CORES  # samples per core
P = 128
CG = 8  # node-tiles (of 128 nodes) gathered per indirect DMA

_cache = {}


def _build(reps: int = 1):
    """Build+compile the SPMD program. reps>1 chains the whole network
    multiple times (identical results; used for wall-clock HW timing)."""
    import concourse.tile as tile
    from concourse import bacc, mybir

    nc = bacc.Bacc("TRN2", target_bir_lowering=False, debug=False,
                   num_devices=N_CORES)
    f32 = mybir.dt.float32
    i32 = mybir.dt.int32

    prev = [IN_DIM] + DIMS[:-1]
    t0 = nc.dram_tensor("t0", [IN_DIM, BPC], f32, kind="ExternalInput")
    t1 = nc.dram_tensor("t1", [DIMS[0], BPC], f32)
    t2 = nc.dram_tensor("t2", [DIMS[1], BPC], f32)
    tables = [t0, t1, t2]
    knn_d, w_d, b_d = [], [], []
    for l, d in enumerate(DIMS):
        nt = d // P
        knn_d.append(nc.dram_tensor(f"knn{l}", [P, nt * K], i32,
                                    kind="ExternalInput"))
        w_d.append(nc.dram_tensor(f"w{l}", [P, nt * K], f32,
                                  kind="ExternalInput"))
        b_d.append(nc.dram_tensor(f"b{l}", [P, nt], f32,
                                  kind="ExternalInput"))
    fcw_d = nc.dram_tensor("fcw", [P, (DIMS[-1] // P) * OUT_DIM], f32,
                           kind="ExternalInput")
    out_d = nc.dram_tensor("out", [BPC, OUT_DIM], f32, kind="ExternalOutput")

    import concourse.bass as bass
    from concourse.bass import IndirectOffsetOnAxis

    with tile.TileContext(nc) as tc:
        with (
            tc.tile_pool(name="const", bufs=1) as cpool,
            tc.tile_pool(name="gath", bufs=3) as gpool,
            tc.tile_pool(name="sums", bufs=3) as spool,
            tc.tile_pool(name="acts", bufs=3) as apool,
            tc.tile_pool(name="psum", bufs=1, space="PSUM") as ppool,
        ):
            # Stage all small tables into persistent SBUF tiles.
            knn_sb, w_sb, b_sb = [], [], []
            for l, d in enumerate(DIMS):
                nt = d // P
                ks = cpool.tile([P, nt * K], i32, tag=f"knn{l}")
                ws = cpool.tile([P, nt * K], f32, tag=f"w{l}")
                bs = cpool.tile([P, nt], f32, tag=f"b{l}")
                nc.sync.dma_start(out=ks[:], in_=knn_d[l].ap())
                nc.sync.dma_start(out=ws[:], in_=w_d[l].ap())
                nc.sync.dma_start(out=bs[:], in_=b_d[l].ap())
                knn_sb.append(ks)
                w_sb.append(ws)
                b_sb.append(bs)
            fcw_sb = cpool.tile([P, (DIMS[-1] // P) * OUT_DIM], f32, tag="fcw")
            nc.sync.dma_start(out=fcw_sb[:], in_=fcw_d.ap())
            out2 = cpool.tile([P, DIMS[-1] // P, BPC], f32, tag="out2")

            def emit_net():
                for l, d in enumerate(DIMS):
                    nt = d // P
                    table = tables[l]
                    for g0 in range(0, nt, CG):
                        g = min(CG, nt - g0)
                        gt = gpool.tile([P, CG * K, BPC], f32, tag="G")
                        # HW contract: one index per partition per indirect
                        # DMA (multi-index offset tiles mis-expand on HW).
                        for j in range(g * K):
                            nc.gpsimd.indirect_dma_start(
                                out=gt[:, j, :],
                                out_offset=None,
                                in_=table.ap(),
                                in_offset=IndirectOffsetOnAxis(
                                    ap=knn_sb[l][:, g0 * K + j : g0 * K + j + 1],
                                    axis=0,
                                ),
                            )
                        # gt[p, (c k), b] *= w[node, k] (broadcast over batch)
                        nc.vector.tensor_mul(
                            out=gt[:, : g * K, :],
                            in0=gt[:, : g * K, :],
                            in1=w_sb[l][:, g0 * K : (g0 + g) * K]
                            .unsqueeze(-1)
                            .to_broadcast([P, g * K, BPC]),
                        )
                        st = spool.tile([P, CG, BPC], f32, tag="S")
                        nc.vector.tensor_reduce(
                            out=st[:, :g, :],
                            in_=gt[:, : g * K, :].rearrange(
                                "p (c k) b -> p c b k", k=K
                            ),
                            axis=mybir.AxisListType.X,
                            op=mybir.AluOpType.add,
                        )
                        if l < 2:
                            at = apool.tile([P, CG, BPC], f32, tag="A")
                        else:
                            at = out2
                        for c in range(g):
                            oc = g0 + c if l == 2 else c
                            nc.scalar.activation(
                                out=at[:, oc, :],
                                in_=st[:, c, :],
                                func=mybir.ActivationFunctionType.Relu,
                                bias=b_sb[l][:, g0 + c : g0 + c + 1],
                                scale=1.0,
                            )
                        if l < 2:
                            dst = tables[l + 1][
                                g0 * P : (g0 + g) * P, :
                            ].rearrange("(c p) b -> p c b", p=P)
                            nc.sync.dma_start(out=dst, in_=at[:, :g, :])
                    if l < 2:
                        tc.strict_bb_all_engine_barrier()

                # FC head: ps[b, j] = sum_nodes out2[node, b] * fcwT[node, j]
                ps = ppool.tile([BPC, OUT_DIM], f32, tag="ps")
                nnt = DIMS[-1] // P
                for t in range(nnt):
                    nc.tensor.matmul(
                        out=ps[:],
                        lhsT=out2[:, t, :],
                        rhs=fcw_sb[:, t * OUT_DIM : (t + 1) * OUT_DIM],
                        start=(t == 0),
                        stop=(t == nnt - 1),
                    )
                fin = apool.tile([BPC, OUT_DIM], f32, tag="fin")
                nc.vector.tensor_copy(out=fin[:], in_=ps[:])
                nc.sync.dma_start(out=out_d.ap(), in_=fin[:])

            for r in range(reps):
                if r:
                    tc.strict_bb_all_engine_barrier()
                emit_net()

    nc.compile()
    return nc


def _prep_inputs(inputs):
    """Host-side layout preprocessing. Returns per-core input maps."""
    x = np.asarray(inputs["x"], dtype=np.float32)
    xT = np.ascontiguousarray(x.T)  # (IN_DIM, B)
    common = {}
    for l, d in enumerate(DIMS):
        nt = d // P
        knn = np.asarray(inputs[f"knn{l}"], dtype=np.int32)
        w = np.asarray(inputs[f"w{l}"], dtype=np.float32)
        b = np.asarray(inputs[f"b{l}"], dtype=np.float32).reshape(d)
        common[f"knn{l}"] = np.ascontiguousarray(
            knn.reshape(nt, P, K).transpose(1, 0, 2).reshape(P, nt * K)
        )
        common[f"w{l}"] = np.ascontiguousarray(
            w.reshape(nt, P, K).transpose(1, 0, 2).reshape(P, nt * K)
        )
        common[f"b{l}"] = np.ascontiguousarray(b.reshape(nt, P).T)
    fcw = np.asarray(inputs["fc_w"], dtype=np.float32)  # (3, 2048)
    nnt = DIMS[-1] // P
    common["fcw"] = np.ascontiguousarray(
        fcw.T.reshape(nnt, P, OUT_DIM).transpose(1, 0, 2).reshape(P, nnt * OUT_DIM)
    )
    in_maps = []
    for m in range(N_CORES):
        im = dict(common)
        im["t0"] = np.ascontiguousarray(xT[:, m * BPC : (m + 1) * BPC])
        in_maps.append(im)
    return in_maps


def kernel(**inputs) -> np.ndarray:
    from concourse.bass_utils import run_bass_kernel_spmd

    reps = int(os.environ.get("KERNEL_REPS", "1"))
    key = ("nc", reps)
    if key not in _cache:
        _cache[key] = _build(reps)
    nc = _cache[key]

    in_maps = _prep_inputs(inputs)
    res = run_bass_kernel_spmd(nc, in_maps, list(range(N_CORES)))
    if res.exec_time_ns is not None:
        print(f"HW exec time: {res.exec_time_ns} ns")
    out = np.concatenate([r["out"] for r in res.results], axis=0)
    fc_b = np.asarray(inputs["fc_b"], dtype=np.float32)
    return (out + fc_b[None, :]).astype(np.float32)
